# revision 18
# baseline (speedup 1.0000x reference)
"""Trainium2 Bass kernel for nn_CrossOutLayer_2 (dense pairwise MLP).

o[b,n,m] = sum_e W2[e] * gelu(hx[b,n,e] + hy[b,m,e] + b1[e]) + b2
  hx = x0 @ W1[:D] + x @ W1[D:2D],  hy = y @ W1[2D:]

Sharded over (b, n1) across 8 cores: each core owns 128 rows of the
(b*512+n1) index and the full m range. MLP weights replicated.

Per-core dataflow (e=128 on partitions):
  PE:  hxT = Wa.T@x0T + Wb.T@xT ; hyT = Wc.T@yT          (pre-GEMMs)
  DVE: s[:, (n,m)] = hyT + (hxT[:,n]+b1)   (tensor_scalar bcast, fp16 2x mode)
  ACT: g = gelu(s) in n-batches of [8,16,...,16,8] (tapered fill/tail) -> fp16
  PE:  out[m, 2n:2n+2] = g[:, n, 128m-chunk].T @ [W2_hi|W2_lo]
       (fp16 data-stationary matmuls, FWL; W2 split hi/lo recovers ~fp32 dot)
  DVE: merge hi+lo + b2 in two n-halves ; DMA out ; host transposes [m,n]->[n,m]

Measured ~70-75us/core on HW (ACT gelu roofline ~61-66us); rel err ~3.9e-4.
"""

import sys

sys.path.insert(0, "/opt/trn_rl_repo")

import numpy as np

B, N1, N2, D = 2, 512, 512, 128
NCORES = 8
ROWS = B * N1 // NCORES  # 128 (b,n1)-rows per core
MCH = N2 // D            # 4 m-chunks of 128
NCHUNK = 16              # n-values per ACT batch
NB = ROWS // NCHUNK      # 8 batches

_cache = {}


def _build(repeat=1, do_adds=True, do_act=True, do_pe=True, s_f16=True, nchunk=NCHUNK,
           act_func="gelu", bufs=3, g_f32=False, taper=True, split_out=True,
           accum_w2=False, gp_adds=0):
    key = ("nc", repeat, do_adds, do_act, do_pe, s_f16, nchunk, act_func, bufs,
           g_f32, taper, split_out, accum_w2, gp_adds)
    if key in _cache:
        return _cache[key]
    import concourse.bacc as bacc
    import concourse.mybir as mybir
    import concourse.tile as tile

    f32 = mybir.dt.float32
    f16 = mybir.dt.float16
    sdt = f16 if s_f16 else f32
    gdt = f32 if g_f32 else f16
    if taper:
        mid = (ROWS - nchunk) // nchunk
        chunks = [nchunk // 2] + [nchunk] * mid + [nchunk // 2]
    else:
        chunks = [nchunk] * (ROWS // nchunk)
    assert sum(chunks) == ROWS

    nc = bacc.Bacc("TRN2", target_bir_lowering=False, debug=False)
    x0T = nc.dram_tensor("x0T", [D, ROWS], f32, kind="ExternalInput")
    xT = nc.dram_tensor("xT", [D, ROWS], f32, kind="ExternalInput")
    yT = nc.dram_tensor("yT", [D, N2], f32, kind="ExternalInput")
    Wa = nc.dram_tensor("Wa", [D, D], f32, kind="ExternalInput")
    Wb = nc.dram_tensor("Wb", [D, D], f32, kind="ExternalInput")
    Wc = nc.dram_tensor("Wc", [D, D], f32, kind="ExternalInput")
    b1c = nc.dram_tensor("b1c", [D, 1], f32, kind="ExternalInput")
    w2hl = nc.dram_tensor("w2hl", [D, 2], gdt, kind="ExternalInput")
    b2c = nc.dram_tensor("b2c", [D, 1], f32, kind="ExternalInput")
    # outT[m_within_chunk, mc*ROWS + n] = o[n, mc*128 + m]
    outT = nc.dram_tensor("outT", [D, MCH * ROWS], f32, kind="ExternalOutput")

    with tile.TileContext(nc) as tc:
        with (
            tc.tile_pool(name="const", bufs=1) as cpool,
            tc.tile_pool(name="work", bufs=bufs) as wpool,
            tc.tile_pool(name="psum", bufs=1, space="PSUM") as pspool,
        ):

            def body():
                x0T_sb = cpool.tile([D, ROWS], f32, name="x0T_sb", tag="x0T_sb")
                nc.sync.dma_start(x0T_sb[:], x0T[:])
                xT_sb = cpool.tile([D, ROWS], f32, name="xT_sb", tag="xT_sb")
                nc.sync.dma_start(xT_sb[:], xT[:])
                yT_sb = cpool.tile([D, N2], f32, name="yT_sb", tag="yT_sb")
                nc.sync.dma_start(yT_sb[:, : N2 // 2], yT[:, : N2 // 2])
                nc.sync.dma_start(yT_sb[:, N2 // 2 :], yT[:, N2 // 2 :])
                Wa_sb = cpool.tile([D, D], f32, name="Wa_sb", tag="Wa_sb")
                nc.sync.dma_start(Wa_sb[:], Wa[:])
                Wb_sb = cpool.tile([D, D], f32, name="Wb_sb", tag="Wb_sb")
                nc.sync.dma_start(Wb_sb[:], Wb[:])
                Wc_sb = cpool.tile([D, D], f32, name="Wc_sb", tag="Wc_sb")
                nc.sync.dma_start(Wc_sb[:], Wc[:])
                b1_sb = cpool.tile([D, 1], f32, name="b1_sb", tag="b1_sb")
                nc.sync.dma_start(b1_sb[:], b1c[:])
                w2_sb = cpool.tile([D, 2], gdt, name="w2_sb", tag="w2_sb")
                nc.sync.dma_start(w2_sb[:], w2hl[:])
                b2_sb = cpool.tile([D, 1], f32, name="b2_sb", tag="b2_sb")
                nc.sync.dma_start(b2_sb[:], b2c[:])

                # hxT[e, n] = Wa.T @ x0T + Wb.T @ xT, then +b1 on evac
                hx_ps = pspool.tile([D, ROWS], f32, name="hx_ps", tag="hx")
                nc.tensor.matmul(
                    hx_ps[:], Wa_sb[:], x0T_sb[:], start=True, stop=False
                )
                nc.tensor.matmul(hx_ps[:], Wb_sb[:], xT_sb[:], start=False, stop=True)
                hxb_sb = cpool.tile([D, ROWS], f32, name="hxb_sb", tag="hxb_sb")
                nc.vector.tensor_scalar_add(
                    out=hxb_sb[:], in0=hx_ps[:], scalar1=b1_sb[:]
                )

                # hyT[e, m] = Wc.T @ yT
                hy_ps = pspool.tile([D, N2], f32, name="hy_ps", tag="hy")
                nc.tensor.matmul(hy_ps[:], Wc_sb[:], yT_sb[:], start=True, stop=True)
                hyT_sb = cpool.tile([D, N2], sdt, name="hyT_sb", tag="hyT_sb")
                nc.vector.tensor_copy(hyT_sb[:], hy_ps[:])

                outp_w = ROWS if accum_w2 else 2 * ROWS
                outp = [
                    pspool.tile(
                        [D, outp_w],
                        mybir.dt.float32,
                        tag=f"outp{mc}",
                        name=f"outp{mc}",
                    )
                    for mc in range(MCH)
                ]

                o_sb = cpool.tile([D, MCH * ROWS], f32, name="o_sb", tag="o_sb")
                t_sb = cpool.tile([D, MCH * ROWS], f32, name="t_sb", tag="t_sb")
                hi_sb = cpool.tile([D, MCH * ROWS], f32, name="hi_sb", tag="hi_sb")

                def emit_merge(n_lo, n_hi):
                    # evac psum for n in [n_lo, n_hi), +b2, DMA out
                    w = n_hi - n_lo
                    for mc in range(MCH):
                        lo0 = mc * ROWS + n_lo
                        if accum_w2:
                            nc.vector.tensor_scalar_add(
                                out=o_sb[:, lo0 : lo0 + w],
                                in0=outp[mc][:, n_lo:n_hi],
                                scalar1=b2_sb[:],
                            )
                        else:
                            r = outp[mc].rearrange("p (n two) -> p n two", two=2)
                            nc.vector.tensor_copy(
                                hi_sb[:, lo0 : lo0 + w], r[:, n_lo:n_hi, 0]
                            )
                            nc.vector.tensor_add(
                                t_sb[:, lo0 : lo0 + w],
                                hi_sb[:, lo0 : lo0 + w],
                                r[:, n_lo:n_hi, 1],
                            )
                            nc.vector.tensor_scalar_add(
                                out=o_sb[:, lo0 : lo0 + w],
                                in0=t_sb[:, lo0 : lo0 + w],
                                scalar1=b2_sb[:],
                            )
                        nc.sync.dma_start(
                            outT[:, lo0 : lo0 + w], o_sb[:, lo0 : lo0 + w]
                        )

                n_start = 0
                merged = 0
                for ci, cw in enumerate(chunks):
                    s = wpool.tile([D, nchunk * N2], sdt, tag="s", name="s")
                    if do_adds:
                        for j in range(cw):
                            n = n_start + j
                            eng = (
                                nc.gpsimd
                                if cw - 1 - j < gp_adds
                                else nc.vector
                            )
                            eng.tensor_scalar_add(
                                out=s[:, j * N2 : (j + 1) * N2],
                                in0=hyT_sb[:],
                                scalar1=hxb_sb[:, n : n + 1],
                            )
                    else:
                        nc.vector.tensor_copy(s[:, :N2], hyT_sb[:])
                    g = wpool.tile([D, nchunk * N2], gdt, tag="g", name="g")
                    if do_act:
                        af = (mybir.ActivationFunctionType.Gelu
                              if act_func == "gelu"
                              else mybir.ActivationFunctionType.Identity)
                        nc.scalar.activation(g[:, : cw * N2], s[:, : cw * N2], af)
                    else:
                        nc.scalar.copy(g[:, :N2], s[:, :N2])
                    if do_pe:
                        for j in range(cw):
                            n = n_start + j
                            for mc in range(MCH):
                                gsl = g[:, j * N2 + mc * D : j * N2 + (mc + 1) * D]
                                if accum_w2:
                                    nc.tensor.matmul(
                                        outp[mc][:, n : n + 1],
                                        gsl,
                                        w2_sb[:, 0:1],
                                        start=True,
                                        stop=False,
                                    )
                                    nc.tensor.matmul(
                                        outp[mc][:, n : n + 1],
                                        gsl,
                                        w2_sb[:, 1:2],
                                        start=False,
                                        stop=True,
                                    )
                                else:
                                    nc.tensor.matmul(
                                        outp[mc][:, 2 * n : 2 * n + 2],
                                        gsl,
                                        w2_sb[:],
                                        start=True,
                                        stop=True,
                                    )
                    n_start += cw
                    if do_pe and split_out and merged == 0 and n_start >= ROWS // 2:
                        emit_merge(0, n_start)
                        merged = n_start

                if do_pe:
                    emit_merge(merged, ROWS)
                else:
                    nc.vector.tensor_copy(o_sb[:, :N2], hyT_sb[:])
                    nc.sync.dma_start(outT[:], o_sb[:])

            if repeat == 1:
                body()
            else:
                with tc.For_i(
                    0, repeat, 1, hint_engines=(mybir.EngineType.PE,)
                ):
                    body()

    nc.compile()
    _cache[key] = nc
    return nc


def _prep_in_maps(x0, x, y, W1, b1, W2, b2, g_f32=False):
    x0 = np.asarray(x0, np.float32)
    x = np.asarray(x, np.float32)
    y = np.asarray(y, np.float32)
    W1 = np.asarray(W1, np.float32)
    b1 = np.asarray(b1, np.float32)
    W2 = np.asarray(W2, np.float32)
    b2 = np.asarray(b2, np.float32)

    w2f = W2[:, 0]
    wdt = np.float32 if g_f32 else np.float16
    w2_hi = w2f.astype(wdt)
    w2_lo = (w2f - w2_hi.astype(np.float32)).astype(wdt)
    w2hl = np.ascontiguousarray(np.stack([w2_hi, w2_lo], axis=1))
    b1c = np.ascontiguousarray(b1.reshape(D, 1))
    b2c = np.full((D, 1), b2[0], np.float32)
    Wa = np.ascontiguousarray(W1[:D])
    Wb = np.ascontiguousarray(W1[D : 2 * D])
    Wc = np.ascontiguousarray(W1[2 * D :])

    in_maps = []
    for c in range(NCORES):
        b = c // (N1 // ROWS)
        n0 = (c % (N1 // ROWS)) * ROWS
        in_maps.append(
            {
                "x0T": np.ascontiguousarray(x0[b, n0 : n0 + ROWS].T),
                "xT": np.ascontiguousarray(x[b, n0 : n0 + ROWS].T),
                "yT": np.ascontiguousarray(y[b].T),
                "Wa": Wa,
                "Wb": Wb,
                "Wc": Wc,
                "b1c": b1c,
                "w2hl": w2hl,
                "b2c": b2c,
            }
        )
    return in_maps


def kernel(x0, x, y, W1, b1, W2, b2):
    from concourse.bass_utils import run_bass_kernel_spmd

    nc = _build()
    in_maps = _prep_in_maps(x0, x, y, W1, b1, W2, b2)
    res = run_bass_kernel_spmd(nc, in_maps, list(range(NCORES)))
    kernel.last_result = res

    out = np.empty((B, N1, N2), np.float32)
    for c in range(NCORES):
        o = res.results[c]["outT"]  # [m_within, mc*ROWS + n]
        b = c // (N1 // ROWS)
        n0 = (c % (N1 // ROWS)) * ROWS
        # o[m, mc*ROWS + n] -> out[n, mc*128 + m]
        out[b, n0 : n0 + ROWS] = (
            o.reshape(D, MCH, ROWS).transpose(2, 1, 0).reshape(ROWS, N2)
        )
    return out


kernel.last_result = None
